# revision 38
# baseline (speedup 1.0000x reference)
"""Fused RoPE attention + LayerNorm, Trainium2, 8 NeuronCores (SPMD).

Head-parallel sharding: core c owns head pair (2c, 2c+1) and computes
Q/K/V projections + attention for the FULL sequence (both batches) for
its two heads.  Inputs x_qk / x_v are replicated to every core (DMA,
not collectives); the only collective is a tiny LayerNorm-stats
AllReduce per batch (each core holds 128 of the 1024 columns of attn
output), overlapped with attention / LN of the other batch.

v2 changes vs v1:
 - Score matmuls (K=DH=64) use PE row tiling: head0 on array rows 0-63
   (tile_position (0,0)), head1 on rows 64-127 ((64,0)).  Alternating
   T0/T8 matmuls run concurrently in the two array halves: measured
   132.7 ns/MM vs 447 ns/MM serial-T0.  Also kills the q1/k1 copies.
 - AV matmuls use fp8e4 DoubleRow: V and probs in fp8, contraction 256
   keys per MM (249 ns vs 2x232 ns bf16).
 - Softmax exp split across engines: head0 chunks on Act (fp8 out),
   head1 chunks on DVE via Schraudolph bit-trick exp (tensor_scalar
   f32->int32 = float bits, +-3% max) + GpSimd pass2 convert to fp8.
 - V projection restructured: W_v stationary, x moving (N=512), PE
   transpose to [key, dh] layout (vs N=128 matmuls).
 - LN tail: AllReduce (not AllGather), rsqrt via DVE bit-trick seed +
   2 Newton steps (no Act Sqrt table switch), batch-0 LN emitted
   in-stream overlapped with batch-1's collective.
"""
import sys
import types
import os
import numpy as np
from contextlib import ExitStack

for _p in ("/opt/trn_rl_repo",):
    if _p not in sys.path:
        sys.path.append(_p)

# NTFF profile hook shim: lets BASS_TRACE=1 work in images whose antenv
# lacks axon_hooks (bass_utils imports it when tracing under axon).
if "antenv.axon_hooks" not in sys.modules:
    _hooks = types.ModuleType("antenv.axon_hooks")
    _HOOK = [None]
    _hooks.set_axon_ntff_profile_hook = lambda h: _HOOK.__setitem__(0, h)
    _hooks.get_axon_ntff_profile_hook = lambda: _HOOK[0]
    sys.modules["antenv.axon_hooks"] = _hooks
    try:
        from trn_agent_boot.trn_boot import _ntff_profile_via_ctypes

        _HOOK[0] = _ntff_profile_via_ctypes("/opt/axon/libaxon_pjrt.so")
    except Exception:
        pass

import concourse.bass as bass  # noqa: E402
import concourse.bacc as bacc  # noqa: E402
import concourse.mybir as mybir  # noqa: E402
import concourse.tile as tile  # noqa: E402
from concourse import bass_utils  # noqa: E402

F32 = mybir.dt.float32
BF16 = mybir.dt.bfloat16
FP8 = mybir.dt.float8e4
I32 = mybir.dt.int32
NP_BF16 = np.dtype(mybir.dt.np(BF16))
AF = mybir.ActivationFunctionType
ALU = mybir.AluOpType
AX = mybir.AxisListType
DR = mybir.MatmulPerfMode.DoubleRow

B, S, D, H, DH = 2, 2048, 1024, 16, 64
NC = 8
R = B * S             # 4096 rows (positions across both batches)
DC = D // 128         # 8 contraction chunks
NSEG = 8              # projection segments of 512 positions
SEGW = R // NSEG      # 512
KTB = 16              # 128-key tiles per batch
NQB = 4               # 512-wide q blocks per batch
VW = 160              # v_sb row width per key tile: h0[0:64] ones@64
                      # pad[65:80) h1[80:144] ones@144 pad[145:160)
LN_EPS = 1e-5
ROPE_BASE = 10000.0

# Schraudolph exp, one pass to bf16 bits: int16 = x*(2^7/ln2)*scale + B
# (truncating f32->int16 convert; int16 pattern == bf16 bits of exp)
SCH_A16 = float(np.float32(0.125 * 2.0 ** 7 / np.log(2.0)))
SCH_B16 = 16250.89      # calibrated: max rel err 3.25%, trunc convert
RSQ_K = float(np.float32(0x5F3759DF))   # rsqrt seed magic as float
I16 = mybir.dt.int16

# chunking of the 16 key tiles per (b, qb) unit: 8 chunks of 2 kt
NCH = 8
CHW = 2 * 512         # psum cols per head-chunk (2 banks)
# exp routing: chunk index -> engine for each head (tunable)
KACT1 = int(os.environ.get("KACT1", "2"))   # h1 chunks also on Act
KDR = os.environ.get("KDR", "0") == "1"     # fp8 DoubleRow AV
KARED = os.environ.get("KARED", "0") == "1"  # AllReduce vs AllGather
KLN0 = os.environ.get("KLN0", "1") == "1"   # batch-0 LN in-stream
# 10: proj only; 15: +scores/exp only; 20: +attention; 26: +collectives;
# 30: full
KSTAGE = int(os.environ.get("KSTAGE", "30"))
PDT = FP8 if KDR else BF16


def _build(flags):
    has_bqk, has_bv, has_gb = flags
    nc = bacc.Bacc("TRN2", target_bir_lowering=False, debug=False,
                   num_devices=NC)

    xqT = nc.dram_tensor("xqT", [NSEG * 128, DC * SEGW], BF16,
                         kind="ExternalInput")
    xvT = nc.dram_tensor("xvT", [NSEG * 128, DC * SEGW], BF16,
                         kind="ExternalInput")
    wq_d = nc.dram_tensor("wq", [D, 128], BF16, kind="ExternalInput")
    wk_d = nc.dram_tensor("wk", [D, 128], BF16, kind="ExternalInput")
    wv_d = nc.dram_tensor("wv", [D, 128], BF16, kind="ExternalInput")
    perm_d = nc.dram_tensor("perm", [128, 128], BF16, kind="ExternalInput")
    ident_d = nc.dram_tensor("ident", [128, 128], BF16, kind="ExternalInput")
    cos_d = nc.dram_tensor("cos", [128, S], BF16, kind="ExternalInput")
    sin_d = nc.dram_tensor("sin", [128, S], BF16, kind="ExternalInput")
    if has_bqk:
        cq_d = nc.dram_tensor("cq", [128, R], F32, kind="ExternalInput")
        ck_d = nc.dram_tensor("ck", [128, R], F32, kind="ExternalInput")
    if has_bv:
        bv_d = nc.dram_tensor("bv", [128, 1], F32, kind="ExternalInput")
    if has_gb:
        gam_d = nc.dram_tensor("gamma", [128, 128], F32, kind="ExternalInput")
        bet_d = nc.dram_tensor("beta", [128, 128], F32, kind="ExternalInput")
    out_d = nc.dram_tensor("out", [R, 128], F32, kind="ExternalOutput")

    es = ExitStack()
    with es:
        tc = es.enter_context(tile.TileContext(nc))
        constp = es.enter_context(tc.tile_pool(name="const", bufs=1))
        qkp = es.enter_context(tc.tile_pool(name="qkp", bufs=1))
        vfp = es.enter_context(tc.tile_pool(name="vfp", bufs=1))
        ptp = es.enter_context(tc.tile_pool(name="ptp", bufs=4))
        attnp = es.enter_context(tc.tile_pool(name="attnp", bufs=1))
        statp = es.enter_context(tc.tile_pool(name="statp", bufs=1))

        # raw dram tensors (not pool tiles): keeps the LN-tail critical
        # section from pre-wait-chaining on the collective's completion.
        st_b = [nc.dram_tensor(f"stb{b}", [128, 32], F32, kind="Internal")
                for b in range(B)]
        st_r = [nc.dram_tensor(f"str{b}", [128, 32] if KARED
                               else [NC * 128, 32], F32,
                               kind="Internal", addr_space="Shared")
                for b in range(B)]

        cos_sb = constp.tile([128, S], BF16, tag="cos")
        sin_sb = constp.tile([128, S], BF16, tag="sin")
        perm_sb = constp.tile([128, 128], BF16, tag="perm")
        ident_sb = constp.tile([128, 128], BF16, tag="ident")

        cq_sb = ck_sb = bv_sb = gam_sb = bet_sb = None
        if has_bqk:
            cq_sb = constp.tile([128, R], F32, tag="cq")
            ck_sb = constp.tile([128, R], F32, tag="ck")
            for hf in range(2):
                sl = slice(hf * 2048, (hf + 1) * 2048)
                nc.sync.dma_start(cq_sb[:, sl], cq_d[:, sl])
                nc.sync.dma_start(ck_sb[:, sl], ck_d[:, sl])
        if has_bv:
            bv_sb = constp.tile([128, 1], F32, tag="bvs")
            nc.sync.dma_start(bv_sb[:], bv_d[:])
        if has_gb:
            gam_sb = constp.tile([128, 128], F32, tag="gam")
            nc.sync.dma_start(gam_sb[:], gam_d[:])
            bet_sb = constp.tile([128, 128], F32, tag="bet")
            nc.sync.dma_start(bet_sb[:], bet_d[:])

        # [dh-of-pair (h0: 0-63, h1: 64-127), b*2048 + s]
        q_sb = qkp.tile([128, R], BF16, tag="q")
        k_sb = qkp.tile([128, R], BF16, tag="k")
        # [key-in-tile, kt*VW + hl*80 + (dh | ones@64)]
        v_sb = vfp.tile([128, 2 * KTB * VW], PDT, tag="v")
        # per-(kt,head) 80-wide blocks; per-head [128, kt, 65] views
        vP = v_sb[:].rearrange("p (blk e) -> p blk e", e=80)
        vKT = v_sb[:].rearrange("p (kt he) -> p kt he", he=VW)
        vH = [vKT[:, :, 0:65], vKT[:, :, 80:145]]
        # [q-in-tile, tt*128 + hl*64 + dh] for row-tile tt
        attn_sb = attnp.tile([128, 32 * 128], F32, tag="attn")
        # [row, tt*2 + (sum|sumsq)]
        stats_sb = statp.tile([128, 64], F32, tag="stats")

        # ---------------- Q/K projections + RoPE ----------------
        pes = ExitStack()
        with pes:
            xqp = pes.enter_context(tc.tile_pool(name="xqp", bufs=3))
            wp = pes.enter_context(tc.tile_pool(name="wp", bufs=1))
            usbp = pes.enter_context(tc.tile_pool(name="usbp", bufs=4))
            stage = pes.enter_context(tc.tile_pool(name="stage", bufs=6))
            pjp = pes.enter_context(
                tc.tile_pool(name="pjp", bufs=6, space="PSUM"))

            def load_w2(t_dram, tg, eng, pool):
                # one 3-D DMA: [dc, p, col] -> [p, dc*128 + col]
                w_sb = pool.tile([128, DC * 128], BF16, tag=tg)
                eng.dma_start(
                    w_sb[:].rearrange("p (dc c) -> p dc c", c=128),
                    t_dram[:].rearrange("(dc p) c -> p dc c", p=128))
                return w_sb

            def load_xseg(pool, src, seg, tg, eng):
                # host pre-tiled: one plain 2-D DMA per segment
                t = pool.tile([128, DC * SEGW], BF16, tag=tg)
                eng.dma_start(t[:], src[seg * 128:(seg + 1) * 128, :])
                return t

            # one hardware DMA queue per issuing engine: spread the input
            # streams across sync/scalar/gpsimd so transfers run in parallel,
            # with first-needed tensors first in each queue.
            wq_sb = load_w2(wq_d, "wq", nc.sync, wp)        # sync q
            xsegs = {}
            xsegs[0] = load_xseg(xqp, xqT, 0, "xq", nc.sync)
            wk_sb = load_w2(wk_d, "wk", nc.scalar, wp)      # act q
            xsegs[1] = load_xseg(xqp, xqT, 1, "xq", nc.scalar)
            nc.gpsimd.dma_start(perm_sb[:], perm_d[:])      # pool q
            nc.gpsimd.dma_start(cos_sb[:], cos_d[:])
            nc.gpsimd.dma_start(sin_sb[:], sin_d[:])
            wv_sb = load_w2(wv_d, "wv", nc.gpsimd, constp)
            nc.gpsimd.dma_start(ident_sb[:], ident_d[:])

            # ones columns of v_sb (cols kt*VW + hl*80 + 64)
            nc.vector.memset(vP[:, :, 64:65], 1.0)

            def proj_u(w_sb, xseg, seg, nm):
                ps_u = pjp.tile([128, SEGW], F32, tag="pj",
                                name=f"psu_{nm}_{seg}")
                for dc in range(DC):
                    nc.tensor.matmul(
                        ps_u[:],
                        w_sb[:, dc * 128:(dc + 1) * 128],
                        xseg[:, dc * SEGW:(dc + 1) * SEGW],
                        start=(dc == 0), stop=(dc == DC - 1))
                return ps_u

            def rope(ps_u, seg, c_sb, dst, nm):
                sl = slice(seg * SEGW, (seg + 1) * SEGW)
                scs = slice((seg % 4) * SEGW, (seg % 4 + 1) * SEGW)
                u_sb = usbp.tile([128, SEGW], BF16, tag="usb",
                                 name=f"usb_{nm}_{seg}")
                nc.scalar.copy(u_sb[:], ps_u[:])                     # Act
                ps_u2 = pjp.tile([128, SEGW], F32, tag="pj",
                                 name=f"psu2_{nm}_{seg}")
                nc.tensor.matmul(ps_u2[:], perm_sb[:], u_sb[:],
                                 start=True, stop=True)
                t1 = stage.tile([128, SEGW], F32, tag="st",
                                name=f"t1_{nm}_{seg}")
                nc.vector.tensor_mul(t1[:], ps_u[:], cos_sb[:, scs])  # DVE
                t2 = stage.tile([128, SEGW], F32, tag="st",
                                name=f"t2_{nm}_{seg}")
                nc.vector.tensor_mul(t2[:], ps_u2[:], sin_sb[:, scs])  # DVE
                if c_sb is None:
                    nc.vector.tensor_add(dst[:, sl], t1[:], t2[:])   # DVE
                else:
                    t3 = stage.tile([128, SEGW], F32, tag="st",
                                    name=f"t3_{nm}_{seg}")
                    nc.vector.tensor_add(t3[:], t1[:], t2[:])
                    nc.vector.tensor_add(dst[:, sl], t3[:], c_sb[:, sl])

            for seg in range(NSEG):
                xseg = xsegs[seg] if seg in xsegs else load_xseg(
                    xqp, xqT, seg, "xq",
                    nc.sync if seg % 2 == 0 else nc.scalar)
                ps_q = proj_u(wq_sb, xseg, seg, "q")
                ps_k = proj_u(wk_sb, xseg, seg, "k")
                rope(ps_q, seg, cq_sb, q_sb, "q")
                rope(ps_k, seg, ck_sb, k_sb, "k")

        # ---------------- V projection ----------------
        ves = ExitStack()
        with ves:
            xvp = ves.enter_context(tc.tile_pool(name="xvp", bufs=2))
            vtp = ves.enter_context(tc.tile_pool(name="vtp", bufs=2))
            pvp = ves.enter_context(
                tc.tile_pool(name="pvp", bufs=2, space="PSUM"))
            trvp = ves.enter_context(
                tc.tile_pool(name="trvp", bufs=2, space="PSUM"))

            def v_proj(xvseg, seg):
                # V^T: wv stationary, x moving -> [128 cols, 512 pos]
                ps_vT = pvp.tile([128, SEGW], F32, tag="pv",
                                 name=f"psvT_{seg}")
                for dc in range(DC):
                    nc.tensor.matmul(
                        ps_vT[:],
                        wv_sb[:, dc * 128:(dc + 1) * 128],
                        xvseg[:, dc * SEGW:(dc + 1) * SEGW],
                        start=(dc == 0), stop=(dc == DC - 1))
                vT_sb = vtp.tile([128, SEGW], BF16, tag="vt",
                                 name=f"vT_{seg}")
                if has_bv:
                    nc.scalar.activation(vT_sb[:], ps_vT[:], AF.Identity,
                                         bias=bv_sb[:, 0:1])         # Act
                else:
                    nc.scalar.copy(vT_sb[:], ps_vT[:])               # Act
                for j in range(4):
                    kt = seg * 4 + j
                    tr = trvp.tile([128, 128], BF16, tag="trv",
                                   name=f"trv_{kt}")
                    nc.tensor.transpose(
                        tr[:], vT_sb[:, j * 128:(j + 1) * 128], ident_sb[:])
                    dst = vP[:, 2 * kt:2 * kt + 2, 0:64]
                    nc.vector.tensor_copy(                           # DVE
                        dst, tr[:].rearrange("p (h d) -> p h d", d=64))

            for seg in range(NSEG):
                xvseg = load_xseg(xvp, xvT, seg, "xv",
                                  nc.gpsimd if seg % 2 == 0 else nc.sync)
                v_proj(xvseg, seg)

        # Full cross-engine fence: attention-phase PSUM tiles reuse the
        # projection pools' banks, and a matmul start=True zeroes its whole
        # 2KB PSUM bank -- without the fence that clobbers proj psum tiles
        # that still have pending readers.
        with tc.tile_critical(name="proj_done"):
            pass

        # ---------------- attention ----------------
        aes = ExitStack()
        with aes:
            scp = aes.enter_context(
                tc.tile_pool(name="scp", bufs=2, space="PSUM"))
            avp = aes.enter_context(
                tc.tile_pool(name="avp", bufs=2, space="PSUM"))
            trp = aes.enter_context(
                tc.tile_pool(name="trp", bufs=1, space="PSUM"))
            atsb = aes.enter_context(tc.tile_pool(name="atsb", bufs=2))
            recp = aes.enter_context(tc.tile_pool(name="recp", bufs=2))
            sqp = aes.enter_context(tc.tile_pool(name="sqp", bufs=2))
            tip = aes.enter_context(tc.tile_pool(name="tip", bufs=2))
            lnsp = aes.enter_context(tc.tile_pool(name="lnsp", bufs=1))
            loup = aes.enter_context(tc.tile_pool(name="loup", bufs=1))

            units = [(b, qb) for b in range(B) for qb in range(NQB)]

            def sch_exp(psx, ptb_i16, base, w):
                """Schraudolph exp psum->bf16 bits in ONE DVE pass: the
                truncating f32->int16 convert of x*A+B yields the bf16
                bit pattern of exp(x*scale)."""
                nc.vector.tensor_scalar(ptb_i16[:, base:base + w],
                                        psx[:, 0:w],
                                        SCH_A16, SCH_B16, ALU.mult, ALU.add)

            def av_jobs_for(u):
                if KDR:
                    return [(hl, pr) for hl in range(2) for pr in range(8)]
                return [(hl, kt) for hl in range(2) for kt in range(KTB)]

            def av_issue(pu, pt_pair, aTs, job):
                pb, pqb = pu
                hl, pr = job
                if aTs[hl] is None:
                    aTs[hl] = avp.tile([65, 512], F32, tag="av",
                                       name=f"aT_{pb}_{pqb}_{hl}")
                if KDR:
                    ktg = pb * KTB + 2 * pr
                    lhs = vH[hl][:, ktg:ktg + 2, :]          # [128, 2, 65]
                    rhs = pt_pair[hl][:, pr * 1024:(pr + 1) * 1024] \
                        .rearrange("p (s n) -> p s n", s=2)  # [128, 2, 512]
                    nc.tensor.matmul(aTs[hl][:], lhs, rhs,
                                     start=(pr == 0), stop=(pr == 7),
                                     skip_group_check=True, perf_mode=DR)
                else:
                    ktg = pb * KTB + pr
                    nc.tensor.matmul(
                        aTs[hl][:], vH[hl][:, ktg:ktg + 1, :],
                        pt_pair[hl][:, pr * 512:(pr + 1) * 512],
                        start=(pr == 0), stop=(pr == KTB - 1),
                        skip_group_check=True)

            def stage1(u, pend):
                """scores + exp of u, with AV matmuls + stage2 of `pend`
                interleaved between score chunks."""
                if u is not None:
                    b, qb = u
                    qsl = slice(b * S + qb * 512, b * S + (qb + 1) * 512)
                    ptA = ptp.tile([128, KTB * 512], PDT, tag="pt",
                                   name=f"ptA_{b}_{qb}")
                    if KDR:
                        ptB = ptp.tile([128, KTB * 512], PDT, tag="pt",
                                       name=f"ptB_{b}_{qb}")
                        ptBv = ptB
                    else:
                        # int16 tile: schraudolph writes bf16 BITS into it
                        ptB = ptp.tile([128, KTB * 512], I16, tag="pt",
                                       name=f"ptB_{b}_{qb}")
                        ptBv = ptB[:].bitcast(BF16)
                    uname = f"{b}_{qb}"
                else:
                    ptA = ptBv = None
                jobs = av_jobs_for(pend[0]) if pend is not None else []
                nj = len(jobs)
                aTs = [None, None]
                for ci in range(NCH):
                    # AV of `pend` in two contiguous per-head blocks: a
                    # DoubleRow accumulation group must not be interrupted
                    # by tile-mode switches (row-tiled score MMs).
                    if pend is not None and ci in (0, NCH // 2):
                        hl = 0 if ci == 0 else 1
                        if hl == 1:
                            if KSTAGE >= 18:
                                stage2(pend[0], 0, aTs[0])
                            else:
                                aT_consume(pend[0], 0, aTs[0])
                        for job in jobs[hl * nj // 2:(hl + 1) * nj // 2]:
                            av_issue(pend[0], pend[1], aTs, job)
                    if u is not None:
                        kt0 = ci * 2
                        psA = scp.tile([128, CHW], F32, tag="sc",
                                       name=f"scA_{uname}_{ci}")
                        psB = scp.tile([128, CHW], F32, tag="sc",
                                       name=f"scB_{uname}_{ci}")
                        for j in range(2):
                            ktb = kt0 + j
                            ksl = slice(b * S + ktb * 128,
                                        b * S + (ktb + 1) * 128)
                            nc.tensor.matmul(
                                psA[:, j * 512:(j + 1) * 512],
                                k_sb[0:64, ksl], q_sb[0:64, qsl],
                                start=True, stop=True,
                                skip_group_check=True, tile_position=(0, 0))
                            nc.tensor.matmul(
                                psB[:, j * 512:(j + 1) * 512],
                                k_sb[64:128, ksl], q_sb[64:128, qsl],
                                start=True, stop=True,
                                skip_group_check=True, tile_position=(64, 0))
                        base = kt0 * 512
                        nc.scalar.activation(ptA[:, base:base + CHW],
                                             psA[:], AF.Exp, scale=0.125)
                        if KDR or ci < KACT1:
                            nc.scalar.activation(ptBv[:, base:base + CHW],
                                                 psB[:], AF.Exp, scale=0.125)
                        else:
                            sch_exp(psB, ptB, base, CHW)
                if pend is not None:
                    if KSTAGE >= 18:
                        stage2(pend[0], 1, aTs[1])
                        if KSTAGE >= 19:
                            stats_u(pend[0])
                    else:
                        aT_consume(pend[0], 1, aTs[1])
                return (ptA, ptBv)

            def aT_consume(u, hl, aT):
                b, qb = u
                aT_sb = atsb.tile([65, 512], BF16, tag="ats",
                                  name=f"ats_{b}_{qb}_{hl}")
                nc.vector.tensor_copy(aT_sb[:], aT[:])               # DVE

            def stage2(u, hl, aT):
                """transpose + normalize -> attn_sb columns."""
                b, qb = u
                aT_sb = atsb.tile([65, 512], BF16, tag="ats",
                                  name=f"ats_{b}_{qb}_{hl}")
                nc.vector.tensor_copy(aT_sb[:], aT[:])               # DVE
                tr = trp.tile([128, 264], BF16, tag="tr",
                              name=f"tr_{b}_{qb}_{hl}")
                for t in range(4):
                    nc.tensor.transpose(
                        tr[:, t * 66: t * 66 + 65],
                        aT_sb[:, t * 128:(t + 1) * 128],
                        ident_sb[0:65, 0:65])
                tr_sb = atsb.tile([128, 260], BF16, tag="trs",
                                  name=f"trs_{b}_{qb}_{hl}")
                nc.vector.tensor_copy(                               # DVE
                    tr_sb[:].rearrange("p (t e) -> p t e", e=65),
                    tr[:].rearrange("p (t e) -> p t e", e=66)[:, :, 0:65])
                rec = recp.tile([128, 4], F32, tag="rec",
                                name=f"rec_{b}_{qb}_{hl}")
                nc.vector.reciprocal(rec[:], tr_sb[:, 64::65])       # DVE
                # normalize: 3 DVE / 3 Act / 2 GpSimd per unit
                NORM_ENG = ("v", "a", "g", "v", "a", "v", "a", "g")
                for t in range(4):
                    tt = b * 16 + qb * 4 + t
                    osl = attn_sb[:, tt * 128 + hl * 64:
                                  tt * 128 + hl * 64 + 64]
                    src = tr_sb[:, t * 65: t * 65 + 64]
                    e = NORM_ENG[hl * 4 + t]
                    if e == "a":
                        nc.scalar.activation(osl, src, AF.Copy,
                                             scale=rec[:, t: t + 1])
                    else:
                        eng = nc.vector if e == "v" else nc.gpsimd
                        eng.tensor_scalar(osl, src,
                                          rec[:, t: t + 1], None, ALU.mult)

            def stats_u(u):
                b, qb = u
                for t in range(4):
                    tt = b * 16 + qb * 4 + t
                    at = attn_sb[:, tt * 128:(tt + 1) * 128]
                    sq = sqp.tile([128, 128], F32, tag="sq",
                                  name=f"sq_{tt}")
                    nc.gpsimd.tensor_mul(sq[:], at, at)              # Pool
                    nc.vector.reduce_sum(                            # DVE
                        stats_sb[:, 2 * tt + 1: 2 * tt + 2], sq[:],
                        axis=AX.X)
                    nc.vector.reduce_sum(                            # DVE
                        stats_sb[:, 2 * tt: 2 * tt + 1], at, axis=AX.X)

            def stats_flush(b):
                nc.sync.dma_start(st_b[b][:],
                                  stats_sb[:, b * 32:(b + 1) * 32])
                nc.gpsimd.collective_compute(
                    "AllReduce" if KARED else "AllGather",
                    ALU.add if KARED else ALU.bypass,
                    ins=[st_b[b][:].opt()], outs=[st_r[b][:].opt()],
                    replica_groups=[list(range(NC))])

            def ln_half(b, lnp, outp):
                tot = lnp.tile([128, 32], F32, tag="tot", name=f"tot{b}")
                if KARED:
                    nc.sync.dma_start(tot[:], st_r[b][:])
                else:
                    tot8 = lnp.tile([128, 8 * 32], F32, tag="tot8",
                                    name=f"tot8{b}")
                    nc.sync.dma_start(
                        tot8[:].rearrange("p (c w) -> p c w", w=32),
                        st_r[b][:].rearrange("(c p) w -> p c w", p=128))
                    nc.vector.tensor_add(tot[:], tot8[:, 0:32],
                                         tot8[:, 32:64])
                    for c in range(2, NC):
                        nc.vector.tensor_add(tot[:], tot[:],
                                             tot8[:, c * 32:(c + 1) * 32])
                nmu = lnp.tile([128, 16], F32, tag="nmu", name=f"nmu{b}")
                nc.vector.tensor_scalar_mul(nmu[:], tot[:, 0::2], -1.0 / D)
                ex2 = lnp.tile([128, 16], F32, tag="ex2", name=f"ex2{b}")
                nc.vector.tensor_scalar_mul(ex2[:], tot[:, 1::2], 1.0 / D)
                var = lnp.tile([128, 16], F32, tag="var", name=f"var{b}")
                nc.vector.tensor_tensor(var[:], nmu[:], nmu[:], ALU.mult)
                nc.vector.tensor_tensor(var[:], ex2[:], var[:], ALU.subtract)
                v2t = lnp.tile([128, 16], F32, tag="v2", name=f"v2{b}")
                nc.vector.tensor_scalar(v2t[:], var[:], 1.0, LN_EPS,
                                        ALU.mult, ALU.add)
                # rsqrt: bit-trick seed + 2 Newton iterations (all DVE)
                cf = lnp.tile([128, 16], F32, tag="cf", name=f"cf{b}")
                nc.vector.tensor_copy(cf[:], v2t[:].bitcast(I32))
                y0i = lnp.tile([128, 16], I32, tag="y0i", name=f"y0i{b}")
                nc.vector.tensor_scalar(y0i[:], cf[:], -0.5, RSQ_K,
                                        ALU.mult, ALU.add)
                y = y0i[:].bitcast(F32)
                tN = lnp.tile([128, 16], F32, tag="tN", name=f"tN{b}")
                for _ in range(2):
                    nc.vector.tensor_tensor(tN[:], y, y, ALU.mult)
                    nc.vector.tensor_tensor(tN[:], tN[:], v2t[:], ALU.mult)
                    nc.vector.tensor_scalar(tN[:], tN[:], -0.5, 1.5,
                                            ALU.mult, ALU.add)
                    nc.vector.tensor_tensor(y, y, tN[:], ALU.mult)
                rstd = lnp.tile([128, 16], F32, tag="rstd", name=f"rs{b}")
                nc.vector.tensor_copy(rstd[:], y)
                mrs = lnp.tile([128, 16], F32, tag="mrs", name=f"mrs{b}")
                nc.vector.tensor_tensor(mrs[:], nmu[:], rstd[:], ALU.mult)
                o_sb = outp.tile([128, 16 * 128], F32, tag="o",
                                 name=f"o_{b}")
                for t in range(16):
                    tt = b * 16 + t
                    osl = o_sb[:, t * 128:(t + 1) * 128]
                    if t % 3 == 0:
                        nc.scalar.activation(                        # Act
                            osl, attn_sb[:, tt * 128:(tt + 1) * 128],
                            AF.Identity, bias=mrs[:, t: t + 1],
                            scale=rstd[:, t: t + 1])
                    else:
                        eng = nc.vector if t % 3 == 1 else nc.gpsimd
                        eng.tensor_scalar(
                            osl, attn_sb[:, tt * 128:(tt + 1) * 128],
                            rstd[:, t: t + 1], mrs[:, t: t + 1],
                            ALU.mult, ALU.add)
                    if has_gb:
                        nc.vector.tensor_tensor(
                            osl, osl, gam_sb[:], ALU.mult)
                        nc.vector.tensor_tensor(
                            osl, osl, bet_sb[:], ALU.add)
                # one 3-D DMA: [p, t, col] -> out rows (b*16+t)*128+p
                nc.sync.dma_start(
                    out_d[b * 2048:(b + 1) * 2048, :].rearrange(
                        "(t p) c -> p t c", p=128),
                    o_sb[:].rearrange("p (t c) -> p t c", c=128))

            def dump_debug():
                # debug: dump raw attn (or zeros) so outputs are produced
                for tt in range(32):
                    o_sb = loup.tile([128, 128], F32, tag="od",
                                     name=f"od_{tt}")
                    if KSTAGE >= 18:
                        nc.vector.tensor_copy(
                            o_sb[:], attn_sb[:, tt * 128:(tt + 1) * 128])
                    else:
                        nc.vector.memset(o_sb[:], 0.0)
                    nc.sync.dma_start(out_d[tt * 128:(tt + 1) * 128, :],
                                      o_sb[:])

            if KSTAGE >= 17:
                pend = None
                for u in units:
                    pt_pair = stage1(u, pend)
                    if KSTAGE >= 26 and pend is not None \
                            and pend[0] == (0, NQB - 1):
                        stats_flush(0)
                    pend = (u, pt_pair)
                stage1(None, pend)
                if KSTAGE >= 26:
                    stats_flush(1)
            elif KSTAGE >= 15:
                # scores + exp only: no AV/stage2/stats
                pend = None
                for u in units:
                    stage1(u, None)
            if KSTAGE >= 30:
                # Both LN halves run in-stream: batch-0's overlaps
                # batch-1's AllGather (gather-0 done long ago), and
                # batch-1's is terminal work -- nothing left in the engine
                # queues to block, so no critical section is needed.
                if KLN0:
                    ln_half(0, lnsp, loup)
                    ln_half(1, lnsp, loup)
                else:
                    with tc.tile_critical(name="ln_tail",
                                          no_gpsimd_drain=True):
                        with tile.TileContext(nc) as tc2:
                            with tc2.tile_pool(name="lnp2", bufs=1) as ln2, \
                                    tc2.tile_pool(name="outp2",
                                                  bufs=1) as ou2:
                                ln_half(0, ln2, ou2)
                                ln_half(1, ln2, ou2)
            else:
                dump_debug()

    nc.compile()
    return nc


_CACHE: dict = {}
LAST_EXEC_NS = None


def _rope_tables():
    half = DH // 2
    inv_freq = 1.0 / (ROPE_BASE ** (np.arange(half, dtype=np.float32) / half))
    t = np.arange(S, dtype=np.float32)
    freqs = t[:, None] * inv_freq[None, :]
    emb = np.concatenate([freqs, freqs], axis=-1)          # [S, DH]
    return np.cos(emb).astype(np.float32), np.sin(emb).astype(np.float32)


def prep_flags(inputs):
    b_qk = np.asarray(inputs["b_qk"], dtype=np.float32)
    b_v = np.asarray(inputs["b_v"], dtype=np.float32)
    gamma = np.asarray(inputs["ln_gamma"], dtype=np.float32)
    beta = np.asarray(inputs["ln_beta"], dtype=np.float32)
    return (bool(np.any(b_qk)), bool(np.any(b_v)),
            bool(np.any(gamma != 1.0) or np.any(beta != 0.0)))


def _perm_mat():
    Pm = np.zeros((128, 128), np.float32)
    for i in range(64):
        Pm[2 * i + 1, 2 * i] = -1.0
        Pm[2 * i, 2 * i + 1] = 1.0
    return Pm


def _prep_in_maps(inputs, flags):
    x_qk = np.asarray(inputs["x_qk"], dtype=np.float32)
    x_v = np.asarray(inputs["x_v"], dtype=np.float32)
    W_qk = np.asarray(inputs["W_qk"], dtype=np.float32)
    b_qk = np.asarray(inputs["b_qk"], dtype=np.float32)
    W_v = np.asarray(inputs["W_v"], dtype=np.float32)
    b_v = np.asarray(inputs["b_v"], dtype=np.float32)
    gamma = np.asarray(inputs["ln_gamma"], dtype=np.float32)
    beta = np.asarray(inputs["ln_beta"], dtype=np.float32)

    Pm = _perm_mat()
    Pm64 = Pm[:DH, :DH]
    cos_all, sin_all = _rope_tables()          # [S, 64]
    cos_in = np.ascontiguousarray(
        np.tile(cos_all.T, (2, 1)).astype(NP_BF16))  # [128, 2048]
    sin_in = np.ascontiguousarray(
        np.tile(sin_all.T, (2, 1)).astype(NP_BF16))

    Wq = W_qk[:, :D]
    Wk = W_qk[:, D:]
    bq = b_qk[:D].reshape(H, DH)
    bk = b_qk[D:].reshape(H, DH)
    bq2 = bq @ Pm64
    bk2 = bk @ Pm64

    def seg_tile(x):
        # [seg*128 + p, dc*SEGW + c] = x[seg*SEGW + c, dc*128 + p]
        return np.ascontiguousarray(
            x.reshape(NSEG, SEGW, DC, 128).transpose(0, 3, 2, 1)
            .reshape(NSEG * 128, DC * SEGW).astype(NP_BF16))

    xqT_np = seg_tile(x_qk.reshape(R, D))
    xvT_np = seg_tile(x_v.reshape(R, D))
    perm_np = np.ascontiguousarray(Pm.astype(NP_BF16))
    ident_np = np.ascontiguousarray(np.eye(128, dtype=NP_BF16))

    in_maps = []
    for c in range(NC):
        cols = slice(c * 128, (c + 1) * 128)
        m = {
            "xqT": xqT_np, "xvT": xvT_np,
            "wq": np.ascontiguousarray(Wq[:, cols].astype(NP_BF16)),
            "wk": np.ascontiguousarray(Wk[:, cols].astype(NP_BF16)),
            "wv": np.ascontiguousarray(W_v[:, cols].astype(NP_BF16)),
            "perm": perm_np, "ident": ident_np,
            "cos": cos_in, "sin": sin_in,
        }
        if flags[0]:
            # additive post-RoPE bias tables for this head pair
            def fold(bh, bh2):
                rows = [bh[2 * c + hl][:, None] * cos_all.T
                        + bh2[2 * c + hl][:, None] * sin_all.T
                        for hl in range(2)]          # each [64, S]
                return np.ascontiguousarray(
                    np.tile(np.vstack(rows), (1, 2)).astype(np.float32))
            m["cq"] = fold(bq, bq2)
            m["ck"] = fold(bk, bk2)
        if flags[1]:
            m["bv"] = np.ascontiguousarray(
                b_v[c * 128:(c + 1) * 128].astype(np.float32)
                .reshape(128, 1))
        if flags[2]:
            m["gamma"] = np.ascontiguousarray(np.broadcast_to(
                gamma[c * 128:(c + 1) * 128], (128, 128)).astype(np.float32))
            m["beta"] = np.ascontiguousarray(np.broadcast_to(
                beta[c * 128:(c + 1) * 128], (128, 128)).astype(np.float32))
        in_maps.append(m)
    return in_maps


def kernel(**inputs):
    flags = prep_flags(inputs)
    if flags not in _CACHE:
        _CACHE[flags] = _build(flags)
    nc = _CACHE[flags]
    in_maps = _prep_in_maps(inputs, flags)
    res = bass_utils.run_bass_kernel_spmd(
        nc, in_maps, core_ids=list(range(NC)))
    global LAST_EXEC_NS
    LAST_EXEC_NS = res.exec_time_ns
    out = np.empty((R, D), np.float32)
    for c in range(NC):
        out[:, c * 128:(c + 1) * 128] = np.asarray(
            res.results[c]["out"], dtype=np.float32)
    return out.reshape(B, S, D)


# revision 42
# speedup vs baseline: 1.1616x; 1.1616x over previous
"""Fused RoPE attention + LayerNorm, Trainium2, 8 NeuronCores (SPMD).

Head-parallel sharding: core c owns head pair (2c, 2c+1) and computes
Q/K/V projections + attention for the FULL sequence (both batches) for
its two heads.  Inputs x_qk / x_v are replicated to every core (DMA,
not collectives); the only collective is a tiny LayerNorm-stats
AllReduce per batch (each core holds 128 of the 1024 columns of attn
output), overlapped with attention / LN of the other batch.

v2 changes vs v1:
 - Score matmuls (K=DH=64) use PE row tiling: head0 on array rows 0-63
   (tile_position (0,0)), head1 on rows 64-127 ((64,0)).  Alternating
   T0/T8 matmuls run concurrently in the two array halves: measured
   132.7 ns/MM vs 447 ns/MM serial-T0.  Also kills the q1/k1 copies.
 - AV matmuls use fp8e4 DoubleRow: V and probs in fp8, contraction 256
   keys per MM (249 ns vs 2x232 ns bf16).
 - Softmax exp split across engines: head0 chunks on Act (fp8 out),
   head1 chunks on DVE via Schraudolph bit-trick exp (tensor_scalar
   f32->int32 = float bits, +-3% max) + GpSimd pass2 convert to fp8.
 - V projection restructured: W_v stationary, x moving (N=512), PE
   transpose to [key, dh] layout (vs N=128 matmuls).
 - LN tail: AllReduce (not AllGather), rsqrt via DVE bit-trick seed +
   2 Newton steps (no Act Sqrt table switch), batch-0 LN emitted
   in-stream overlapped with batch-1's collective.
"""
import sys
import types
import os
import numpy as np
from contextlib import ExitStack

for _p in ("/opt/trn_rl_repo",):
    if _p not in sys.path:
        sys.path.append(_p)

# NTFF profile hook shim: lets BASS_TRACE=1 work in images whose antenv
# lacks axon_hooks (bass_utils imports it when tracing under axon).
if "antenv.axon_hooks" not in sys.modules:
    _hooks = types.ModuleType("antenv.axon_hooks")
    _HOOK = [None]
    _hooks.set_axon_ntff_profile_hook = lambda h: _HOOK.__setitem__(0, h)
    _hooks.get_axon_ntff_profile_hook = lambda: _HOOK[0]
    sys.modules["antenv.axon_hooks"] = _hooks
    try:
        from trn_agent_boot.trn_boot import _ntff_profile_via_ctypes

        _HOOK[0] = _ntff_profile_via_ctypes("/opt/axon/libaxon_pjrt.so")
    except Exception:
        pass

import concourse.bass as bass  # noqa: E402
import concourse.bacc as bacc  # noqa: E402
import concourse.mybir as mybir  # noqa: E402
import concourse.tile as tile  # noqa: E402
from concourse import bass_utils  # noqa: E402

F32 = mybir.dt.float32
BF16 = mybir.dt.bfloat16
FP8 = mybir.dt.float8e4
I32 = mybir.dt.int32
NP_BF16 = np.dtype(mybir.dt.np(BF16))
AF = mybir.ActivationFunctionType
ALU = mybir.AluOpType
AX = mybir.AxisListType
DR = mybir.MatmulPerfMode.DoubleRow

B, S, D, H, DH = 2, 2048, 1024, 16, 64
NC = 8
R = B * S             # 4096 rows (positions across both batches)
DC = D // 128         # 8 contraction chunks
NSEG = 8              # projection segments of 512 positions
SEGW = R // NSEG      # 512
KTB = 16              # 128-key tiles per batch
NQB = 4               # 512-wide q blocks per batch
VW = 160              # v_sb row width per key tile: h0[0:64] ones@64
                      # pad[65:80) h1[80:144] ones@144 pad[145:160)
LN_EPS = 1e-5
ROPE_BASE = 10000.0

# Schraudolph exp, one pass to bf16 bits: int16 = x*(2^7/ln2)*scale + B
# (truncating f32->int16 convert; int16 pattern == bf16 bits of exp)
SCH_A16 = float(np.float32(0.125 * 2.0 ** 7 / np.log(2.0)))
SCH_B16 = 16250.89      # calibrated: max rel err 3.25%, trunc convert
RSQ_K = float(np.float32(0x5F3759DF))   # rsqrt seed magic as float
I16 = mybir.dt.int16

# chunking of the 16 key tiles per (b, qb) unit: 8 chunks of 2 kt
NCH = 8
CHW = 2 * 512         # psum cols per head-chunk (2 banks)
# exp routing: chunk index -> engine for each head (tunable)
KACT1 = int(os.environ.get("KACT1", "2"))   # h1 chunks also on Act
KDR = os.environ.get("KDR", "0") == "1"     # fp8 DoubleRow AV
KARED = os.environ.get("KARED", "0") == "1"  # AllReduce vs AllGather
KLN0 = os.environ.get("KLN0", "1") == "1"   # batch-0 LN in-stream
# 10: proj only; 15: +scores/exp only; 20: +attention; 26: +collectives;
# 30: full
KSTAGE = int(os.environ.get("KSTAGE", "30"))
PDT = FP8 if KDR else BF16


def _build(flags):
    has_bqk, has_bv, has_gb = flags
    nc = bacc.Bacc("TRN2", target_bir_lowering=False, debug=False,
                   num_devices=NC)

    xqT = nc.dram_tensor("xqT", [NSEG * 128, DC * SEGW], BF16,
                         kind="ExternalInput")
    xvT = nc.dram_tensor("xvT", [NSEG * 128, DC * SEGW], BF16,
                         kind="ExternalInput")
    wq_d = nc.dram_tensor("wq", [D, 128], BF16, kind="ExternalInput")
    wk_d = nc.dram_tensor("wk", [D, 128], BF16, kind="ExternalInput")
    wv_d = nc.dram_tensor("wv", [D, 128], BF16, kind="ExternalInput")
    perm_d = nc.dram_tensor("perm", [128, 128], BF16, kind="ExternalInput")
    ident_d = nc.dram_tensor("ident", [128, 128], BF16, kind="ExternalInput")
    cos_d = nc.dram_tensor("cos", [128, S], BF16, kind="ExternalInput")
    sin_d = nc.dram_tensor("sin", [128, S], BF16, kind="ExternalInput")
    if has_bqk:
        cq_d = nc.dram_tensor("cq", [128, R], F32, kind="ExternalInput")
        ck_d = nc.dram_tensor("ck", [128, R], F32, kind="ExternalInput")
    if has_bv:
        bv_d = nc.dram_tensor("bv", [128, 1], F32, kind="ExternalInput")
    if has_gb:
        gam_d = nc.dram_tensor("gamma", [128, 128], F32, kind="ExternalInput")
        bet_d = nc.dram_tensor("beta", [128, 128], F32, kind="ExternalInput")
    out_d = nc.dram_tensor("out", [R, 128], F32, kind="ExternalOutput")

    es = ExitStack()
    with es:
        tc = es.enter_context(tile.TileContext(nc))
        constp = es.enter_context(tc.tile_pool(name="const", bufs=1))
        qkp = es.enter_context(tc.tile_pool(name="qkp", bufs=1))
        vfp = es.enter_context(tc.tile_pool(name="vfp", bufs=1))
        ptp = es.enter_context(tc.tile_pool(name="ptp", bufs=4))
        attnp = es.enter_context(tc.tile_pool(name="attnp", bufs=1))
        statp = es.enter_context(tc.tile_pool(name="statp", bufs=1))

        # raw dram tensors (not pool tiles): keeps the LN-tail critical
        # section from pre-wait-chaining on the collective's completion.
        st_b = [nc.dram_tensor(f"stb{b}", [128, 32], F32, kind="Internal")
                for b in range(B)]
        st_r = [nc.dram_tensor(f"str{b}", [128, 32] if KARED
                               else [NC * 128, 32], F32,
                               kind="Internal", addr_space="Shared")
                for b in range(B)]

        cos_sb = constp.tile([128, S], BF16, tag="cos")
        sin_sb = constp.tile([128, S], BF16, tag="sin")
        perm_sb = constp.tile([128, 128], BF16, tag="perm")
        ident_sb = constp.tile([128, 128], BF16, tag="ident")

        cq_sb = ck_sb = bv_sb = gam_sb = bet_sb = None
        if has_bqk:
            cq_sb = constp.tile([128, R], F32, tag="cq")
            ck_sb = constp.tile([128, R], F32, tag="ck")
            for hf in range(2):
                sl = slice(hf * 2048, (hf + 1) * 2048)
                nc.sync.dma_start(cq_sb[:, sl], cq_d[:, sl])
                nc.sync.dma_start(ck_sb[:, sl], ck_d[:, sl])
        if has_bv:
            bv_sb = constp.tile([128, 1], F32, tag="bvs")
            nc.sync.dma_start(bv_sb[:], bv_d[:])
        if has_gb:
            gam_sb = constp.tile([128, 128], F32, tag="gam")
            nc.sync.dma_start(gam_sb[:], gam_d[:])
            bet_sb = constp.tile([128, 128], F32, tag="bet")
            nc.sync.dma_start(bet_sb[:], bet_d[:])

        # [dh-of-pair (h0: 0-63, h1: 64-127), b*2048 + s]
        q_sb = qkp.tile([128, R], BF16, tag="q")
        k_sb = qkp.tile([128, R], BF16, tag="k")
        # [key-in-tile, kt*VW + hl*80 + (dh | ones@64)]
        v_sb = vfp.tile([128, 2 * KTB * VW], PDT, tag="v")
        # per-(kt,head) 80-wide blocks; per-head [128, kt, 65] views
        vP = v_sb[:].rearrange("p (blk e) -> p blk e", e=80)
        vKT = v_sb[:].rearrange("p (kt he) -> p kt he", he=VW)
        vH = [vKT[:, :, 0:65], vKT[:, :, 80:145]]
        # [q-in-tile, tt*128 + hl*64 + dh] for row-tile tt
        attn_sb = attnp.tile([128, 32 * 128], F32, tag="attn")
        # [row, tt*2 + (sum|sumsq)]
        stats_sb = statp.tile([128, 64], F32, tag="stats")

        # ---------------- Q/K projections + RoPE ----------------
        pes = ExitStack()
        with pes:
            xqp = pes.enter_context(tc.tile_pool(name="xqp", bufs=3))
            wp = pes.enter_context(tc.tile_pool(name="wp", bufs=1))
            usbp = pes.enter_context(tc.tile_pool(name="usbp", bufs=4))
            stage = pes.enter_context(tc.tile_pool(name="stage", bufs=6))
            pjp = pes.enter_context(
                tc.tile_pool(name="pjp", bufs=6, space="PSUM"))

            def load_w2(t_dram, tg, eng, pool):
                # one 3-D DMA: [dc, p, col] -> [p, dc*128 + col]
                w_sb = pool.tile([128, DC * 128], BF16, tag=tg)
                eng.dma_start(
                    w_sb[:].rearrange("p (dc c) -> p dc c", c=128),
                    t_dram[:].rearrange("(dc p) c -> p dc c", p=128))
                return w_sb

            def load_xseg(pool, src, seg, tg, eng):
                # host pre-tiled: one plain 2-D DMA per segment
                t = pool.tile([128, DC * SEGW], BF16, tag=tg)
                eng.dma_start(t[:], src[seg * 128:(seg + 1) * 128, :])
                return t

            # one hardware DMA queue per issuing engine: spread the input
            # streams across sync/scalar/gpsimd so transfers run in parallel,
            # with first-needed tensors first in each queue.
            wq_sb = load_w2(wq_d, "wq", nc.sync, wp)        # sync q
            xsegs = {}
            xsegs[0] = load_xseg(xqp, xqT, 0, "xq", nc.sync)
            wk_sb = load_w2(wk_d, "wk", nc.scalar, wp)      # act q
            xsegs[1] = load_xseg(xqp, xqT, 1, "xq", nc.scalar)
            nc.gpsimd.dma_start(perm_sb[:], perm_d[:])      # pool q
            nc.gpsimd.dma_start(cos_sb[:], cos_d[:])
            nc.gpsimd.dma_start(sin_sb[:], sin_d[:])
            wv_sb = load_w2(wv_d, "wv", nc.gpsimd, constp)
            nc.gpsimd.dma_start(ident_sb[:], ident_d[:])

            # ones columns of v_sb (cols kt*VW + hl*80 + 64)
            nc.vector.memset(vP[:, :, 64:65], 1.0)

            def proj_u(w_sb, xseg, seg, nm):
                ps_u = pjp.tile([128, SEGW], F32, tag="pj",
                                name=f"psu_{nm}_{seg}")
                for dc in range(DC):
                    nc.tensor.matmul(
                        ps_u[:],
                        w_sb[:, dc * 128:(dc + 1) * 128],
                        xseg[:, dc * SEGW:(dc + 1) * SEGW],
                        start=(dc == 0), stop=(dc == DC - 1))
                return ps_u

            def rope(ps_u, seg, c_sb, dst, nm):
                sl = slice(seg * SEGW, (seg + 1) * SEGW)
                scs = slice((seg % 4) * SEGW, (seg % 4 + 1) * SEGW)
                u_sb = usbp.tile([128, SEGW], BF16, tag="usb",
                                 name=f"usb_{nm}_{seg}")
                nc.scalar.copy(u_sb[:], ps_u[:])                     # Act
                ps_u2 = pjp.tile([128, SEGW], F32, tag="pj",
                                 name=f"psu2_{nm}_{seg}")
                nc.tensor.matmul(ps_u2[:], perm_sb[:], u_sb[:],
                                 start=True, stop=True)
                t1 = stage.tile([128, SEGW], F32, tag="st",
                                name=f"t1_{nm}_{seg}")
                nc.vector.tensor_mul(t1[:], ps_u[:], cos_sb[:, scs])  # DVE
                t2 = stage.tile([128, SEGW], F32, tag="st",
                                name=f"t2_{nm}_{seg}")
                nc.vector.tensor_mul(t2[:], ps_u2[:], sin_sb[:, scs])  # DVE
                if c_sb is None:
                    nc.vector.tensor_add(dst[:, sl], t1[:], t2[:])   # DVE
                else:
                    t3 = stage.tile([128, SEGW], F32, tag="st",
                                    name=f"t3_{nm}_{seg}")
                    nc.vector.tensor_add(t3[:], t1[:], t2[:])
                    nc.vector.tensor_add(dst[:, sl], t3[:], c_sb[:, sl])

            for seg in range(NSEG):
                xseg = xsegs[seg] if seg in xsegs else load_xseg(
                    xqp, xqT, seg, "xq",
                    nc.sync if seg % 2 == 0 else nc.scalar)
                ps_q = proj_u(wq_sb, xseg, seg, "q")
                ps_k = proj_u(wk_sb, xseg, seg, "k")
                rope(ps_q, seg, cq_sb, q_sb, "q")
                rope(ps_k, seg, ck_sb, k_sb, "k")

        # ---------------- V projection ----------------
        ves = ExitStack()
        with ves:
            xvp = ves.enter_context(tc.tile_pool(name="xvp", bufs=2))
            vtp = ves.enter_context(tc.tile_pool(name="vtp", bufs=2))
            pvp = ves.enter_context(
                tc.tile_pool(name="pvp", bufs=2, space="PSUM"))
            trvp = ves.enter_context(
                tc.tile_pool(name="trvp", bufs=2, space="PSUM"))

            def v_proj(xvseg, seg):
                # V^T: wv stationary, x moving -> [128 cols, 512 pos]
                ps_vT = pvp.tile([128, SEGW], F32, tag="pv",
                                 name=f"psvT_{seg}")
                for dc in range(DC):
                    nc.tensor.matmul(
                        ps_vT[:],
                        wv_sb[:, dc * 128:(dc + 1) * 128],
                        xvseg[:, dc * SEGW:(dc + 1) * SEGW],
                        start=(dc == 0), stop=(dc == DC - 1))
                vT_sb = vtp.tile([128, SEGW], BF16, tag="vt",
                                 name=f"vT_{seg}")
                if has_bv:
                    nc.scalar.activation(vT_sb[:], ps_vT[:], AF.Identity,
                                         bias=bv_sb[:, 0:1])         # Act
                else:
                    nc.scalar.copy(vT_sb[:], ps_vT[:])               # Act
                for j in range(4):
                    kt = seg * 4 + j
                    tr = trvp.tile([128, 128], BF16, tag="trv",
                                   name=f"trv_{kt}")
                    nc.tensor.transpose(
                        tr[:], vT_sb[:, j * 128:(j + 1) * 128], ident_sb[:])
                    dst = vP[:, 2 * kt:2 * kt + 2, 0:64]
                    nc.vector.tensor_copy(                           # DVE
                        dst, tr[:].rearrange("p (h d) -> p h d", d=64))

            for seg in range(NSEG):
                xvseg = load_xseg(xvp, xvT, seg, "xv",
                                  nc.gpsimd if seg % 2 == 0 else nc.sync)
                v_proj(xvseg, seg)

        # Full cross-engine fence: attention-phase PSUM tiles reuse the
        # projection pools' banks, and a matmul start=True zeroes its whole
        # 2KB PSUM bank -- without the fence that clobbers proj psum tiles
        # that still have pending readers.
        with tc.tile_critical(name="proj_done"):
            pass

        # ---------------- attention ----------------
        aes = ExitStack()
        with aes:
            scp = aes.enter_context(
                tc.tile_pool(name="scp", bufs=2, space="PSUM"))
            avp = aes.enter_context(
                tc.tile_pool(name="avp", bufs=2, space="PSUM"))
            trp = aes.enter_context(
                tc.tile_pool(name="trp", bufs=1, space="PSUM"))
            atsb = aes.enter_context(tc.tile_pool(name="atsb", bufs=2))
            recp = aes.enter_context(tc.tile_pool(name="recp", bufs=2))
            sqp = aes.enter_context(tc.tile_pool(name="sqp", bufs=2))
            tip = aes.enter_context(tc.tile_pool(name="tip", bufs=2))
            lnsp = aes.enter_context(tc.tile_pool(name="lnsp", bufs=1))
            loup = aes.enter_context(tc.tile_pool(name="loup", bufs=1))

            units = [(b, qb) for b in range(B) for qb in range(NQB)]

            def sch_exp(psx, ptb_i16, base, w):
                """Schraudolph exp psum->bf16 bits in ONE DVE pass: the
                truncating f32->int16 convert of x*A+B yields the bf16
                bit pattern of exp(x*scale)."""
                nc.vector.tensor_scalar(ptb_i16[:, base:base + w],
                                        psx[:, 0:w],
                                        SCH_A16, SCH_B16, ALU.mult, ALU.add)

            def av_jobs_for(u):
                if KDR:
                    return [(hl, pr) for hl in range(2) for pr in range(8)]
                return [(hl, kt) for hl in range(2) for kt in range(KTB)]

            def av_issue(pu, pt_pair, aTs, job):
                pb, pqb = pu
                hl, pr = job
                if aTs[hl] is None:
                    aTs[hl] = avp.tile([65, 512], F32, tag="av",
                                       name=f"aT_{pb}_{pqb}_{hl}")
                if KDR:
                    ktg = pb * KTB + 2 * pr
                    lhs = vH[hl][:, ktg:ktg + 2, :]          # [128, 2, 65]
                    rhs = pt_pair[hl][:, pr * 1024:(pr + 1) * 1024] \
                        .rearrange("p (s n) -> p s n", s=2)  # [128, 2, 512]
                    nc.tensor.matmul(aTs[hl][:], lhs, rhs,
                                     start=(pr == 0), stop=(pr == 7),
                                     skip_group_check=True, perf_mode=DR)
                else:
                    ktg = pb * KTB + pr
                    nc.tensor.matmul(
                        aTs[hl][:], vH[hl][:, ktg:ktg + 1, :],
                        pt_pair[hl][:, pr * 512:(pr + 1) * 512],
                        start=(pr == 0), stop=(pr == KTB - 1),
                        skip_group_check=True)

            def stage1(u, pend):
                """scores + exp of u, with AV matmuls + stage2 of `pend`
                interleaved between score chunks."""
                if u is not None:
                    b, qb = u
                    qsl = slice(b * S + qb * 512, b * S + (qb + 1) * 512)
                    ptA = ptp.tile([128, KTB * 512], PDT, tag="pt",
                                   name=f"ptA_{b}_{qb}")
                    if KDR:
                        ptB = ptp.tile([128, KTB * 512], PDT, tag="pt",
                                       name=f"ptB_{b}_{qb}")
                        ptBv = ptB
                    else:
                        # int16 tile: schraudolph writes bf16 BITS into it
                        ptB = ptp.tile([128, KTB * 512], I16, tag="pt",
                                       name=f"ptB_{b}_{qb}")
                        ptBv = ptB[:].bitcast(BF16)
                    uname = f"{b}_{qb}"
                else:
                    ptA = ptBv = None
                jobs = av_jobs_for(pend[0]) if pend is not None else []
                nj = len(jobs)
                aTs = [None, None]
                for ci in range(NCH):
                    # AV of `pend` in two contiguous per-head blocks: a
                    # DoubleRow accumulation group must not be interrupted
                    # by tile-mode switches (row-tiled score MMs).
                    if pend is not None and ci in (0, NCH // 2):
                        hl = 0 if ci == 0 else 1
                        if hl == 1:
                            if KSTAGE >= 18:
                                stage2(pend[0], 0, aTs[0])
                            else:
                                aT_consume(pend[0], 0, aTs[0])
                        for job in jobs[hl * nj // 2:(hl + 1) * nj // 2]:
                            av_issue(pend[0], pend[1], aTs, job)
                    if u is not None:
                        kt0 = ci * 2
                        psA = scp.tile([128, CHW], F32, tag="sc",
                                       name=f"scA_{uname}_{ci}")
                        psB = scp.tile([128, CHW], F32, tag="sc",
                                       name=f"scB_{uname}_{ci}")
                        for j in range(2):
                            ktb = kt0 + j
                            ksl = slice(b * S + ktb * 128,
                                        b * S + (ktb + 1) * 128)
                            nc.tensor.matmul(
                                psA[:, j * 512:(j + 1) * 512],
                                k_sb[0:64, ksl], q_sb[0:64, qsl],
                                start=True, stop=True,
                                skip_group_check=True, tile_position=(0, 0))
                            nc.tensor.matmul(
                                psB[:, j * 512:(j + 1) * 512],
                                k_sb[64:128, ksl], q_sb[64:128, qsl],
                                start=True, stop=True,
                                skip_group_check=True, tile_position=(64, 0))
                        base = kt0 * 512
                        nc.scalar.activation(ptA[:, base:base + CHW],
                                             psA[:], AF.Exp, scale=0.125)
                        if KDR or ci < KACT1:
                            nc.scalar.activation(ptBv[:, base:base + CHW],
                                                 psB[:], AF.Exp, scale=0.125)
                        else:
                            sch_exp(psB, ptB, base, CHW)
                if pend is not None:
                    if KSTAGE >= 18:
                        stage2(pend[0], 1, aTs[1])
                        if KSTAGE >= 19:
                            stats_u(pend[0])
                    else:
                        aT_consume(pend[0], 1, aTs[1])
                return (ptA, ptBv)

            def aT_consume(u, hl, aT):
                b, qb = u
                aT_sb = atsb.tile([65, 512], BF16, tag="ats",
                                  name=f"ats_{b}_{qb}_{hl}")
                nc.vector.tensor_copy(aT_sb[:], aT[:])               # DVE

            def stage2(u, hl, aT):
                """transpose + normalize -> attn_sb columns."""
                b, qb = u
                aT_sb = atsb.tile([65, 512], BF16, tag="ats",
                                  name=f"ats_{b}_{qb}_{hl}")
                nc.vector.tensor_copy(aT_sb[:], aT[:])               # DVE
                tr = trp.tile([128, 264], BF16, tag="tr",
                              name=f"tr_{b}_{qb}_{hl}")
                for t in range(4):
                    nc.tensor.transpose(
                        tr[:, t * 66: t * 66 + 65],
                        aT_sb[:, t * 128:(t + 1) * 128],
                        ident_sb[0:65, 0:65])
                tr_sb = atsb.tile([128, 260], BF16, tag="trs",
                                  name=f"trs_{b}_{qb}_{hl}")
                nc.vector.tensor_copy(                               # DVE
                    tr_sb[:].rearrange("p (t e) -> p t e", e=65),
                    tr[:].rearrange("p (t e) -> p t e", e=66)[:, :, 0:65])
                rec = recp.tile([128, 4], F32, tag="rec",
                                name=f"rec_{b}_{qb}_{hl}")
                nc.vector.reciprocal(rec[:], tr_sb[:, 64::65])       # DVE
                # normalize: 6 DVE / 2 Act per unit.  NO GpSimd in the
                # attention phase: the stats collectives block the gpsimd
                # queue until all 8 cores arrive, which would stall any
                # gp compute queued behind them (observed 40us stalls).
                for t in range(4):
                    tt = b * 16 + qb * 4 + t
                    osl = attn_sb[:, tt * 128 + hl * 64:
                                  tt * 128 + hl * 64 + 64]
                    src = tr_sb[:, t * 65: t * 65 + 64]
                    if t == 2:
                        nc.scalar.activation(osl, src, AF.Copy,
                                             scale=rec[:, t: t + 1])
                    else:
                        nc.vector.tensor_scalar(osl, src,
                                                rec[:, t: t + 1], None,
                                                ALU.mult)

            def stats_u(u):
                b, qb = u
                for t in range(4):
                    tt = b * 16 + qb * 4 + t
                    at = attn_sb[:, tt * 128:(tt + 1) * 128]
                    sq = sqp.tile([128, 128], F32, tag="sq",
                                  name=f"sq_{tt}")
                    nc.vector.tensor_mul(sq[:], at, at)              # DVE
                    nc.vector.reduce_sum(                            # DVE
                        stats_sb[:, 2 * tt + 1: 2 * tt + 2], sq[:],
                        axis=AX.X)
                    nc.vector.reduce_sum(                            # DVE
                        stats_sb[:, 2 * tt: 2 * tt + 1], at, axis=AX.X)

            def stats_flush(b):
                nc.sync.dma_start(st_b[b][:],
                                  stats_sb[:, b * 32:(b + 1) * 32])
                nc.gpsimd.collective_compute(
                    "AllReduce" if KARED else "AllGather",
                    ALU.add if KARED else ALU.bypass,
                    ins=[st_b[b][:].opt()], outs=[st_r[b][:].opt()],
                    replica_groups=[list(range(NC))])

            def ln_half(b, lnp, outp):
                tot = lnp.tile([128, 32], F32, tag="tot", name=f"tot{b}")
                if KARED:
                    nc.sync.dma_start(tot[:], st_r[b][:])
                else:
                    tot8 = lnp.tile([128, 8 * 32], F32, tag="tot8",
                                    name=f"tot8{b}")
                    nc.sync.dma_start(
                        tot8[:].rearrange("p (c w) -> p c w", w=32),
                        st_r[b][:].rearrange("(c p) w -> p c w", p=128))
                    # tree reduction: 3 DVE ops instead of 7
                    nc.vector.tensor_add(tot8[:, 0:128], tot8[:, 0:128],
                                         tot8[:, 128:256])
                    nc.vector.tensor_add(tot8[:, 0:64], tot8[:, 0:64],
                                         tot8[:, 64:128])
                    nc.vector.tensor_add(tot[:], tot8[:, 0:32],
                                         tot8[:, 32:64])
                nmu = lnp.tile([128, 16], F32, tag="nmu", name=f"nmu{b}")
                nc.vector.tensor_scalar_mul(nmu[:], tot[:, 0::2], -1.0 / D)
                ex2 = lnp.tile([128, 16], F32, tag="ex2", name=f"ex2{b}")
                nc.vector.tensor_scalar_mul(ex2[:], tot[:, 1::2], 1.0 / D)
                var = lnp.tile([128, 16], F32, tag="var", name=f"var{b}")
                nc.vector.tensor_tensor(var[:], nmu[:], nmu[:], ALU.mult)
                nc.vector.tensor_tensor(var[:], ex2[:], var[:], ALU.subtract)
                v2t = lnp.tile([128, 16], F32, tag="v2", name=f"v2{b}")
                nc.vector.tensor_scalar(v2t[:], var[:], 1.0, LN_EPS,
                                        ALU.mult, ALU.add)
                # rsqrt: bit-trick seed + 2 Newton iterations (all DVE)
                cf = lnp.tile([128, 16], F32, tag="cf", name=f"cf{b}")
                nc.vector.tensor_copy(cf[:], v2t[:].bitcast(I32))
                y0i = lnp.tile([128, 16], I32, tag="y0i", name=f"y0i{b}")
                nc.vector.tensor_scalar(y0i[:], cf[:], -0.5, RSQ_K,
                                        ALU.mult, ALU.add)
                y = y0i[:].bitcast(F32)
                tN = lnp.tile([128, 16], F32, tag="tN", name=f"tN{b}")
                for _ in range(2):
                    nc.vector.tensor_tensor(tN[:], y, y, ALU.mult)
                    nc.vector.tensor_tensor(tN[:], tN[:], v2t[:], ALU.mult)
                    nc.vector.tensor_scalar(tN[:], tN[:], -0.5, 1.5,
                                            ALU.mult, ALU.add)
                    nc.vector.tensor_tensor(y, y, tN[:], ALU.mult)
                rstd = lnp.tile([128, 16], F32, tag="rstd", name=f"rs{b}")
                nc.vector.tensor_copy(rstd[:], y)
                mrs = lnp.tile([128, 16], F32, tag="mrs", name=f"mrs{b}")
                nc.vector.tensor_tensor(mrs[:], nmu[:], rstd[:], ALU.mult)
                o_sb = outp.tile([128, 16 * 128], F32, tag="o",
                                 name=f"o_{b}")
                for t in range(16):
                    tt = b * 16 + t
                    osl = o_sb[:, t * 128:(t + 1) * 128]
                    # gp only for batch 1 (its ops would otherwise queue
                    # behind the blocking gather-1 on the gpsimd queue)
                    if t % 3 == 0:
                        nc.scalar.activation(                        # Act
                            osl, attn_sb[:, tt * 128:(tt + 1) * 128],
                            AF.Identity, bias=mrs[:, t: t + 1],
                            scale=rstd[:, t: t + 1])
                    else:
                        eng = nc.vector if (t % 3 == 1 or b == 0) \
                            else nc.gpsimd
                        eng.tensor_scalar(
                            osl, attn_sb[:, tt * 128:(tt + 1) * 128],
                            rstd[:, t: t + 1], mrs[:, t: t + 1],
                            ALU.mult, ALU.add)
                    if has_gb:
                        nc.vector.tensor_tensor(
                            osl, osl, gam_sb[:], ALU.mult)
                        nc.vector.tensor_tensor(
                            osl, osl, bet_sb[:], ALU.add)
                # one 3-D DMA: [p, t, col] -> out rows (b*16+t)*128+p
                nc.sync.dma_start(
                    out_d[b * 2048:(b + 1) * 2048, :].rearrange(
                        "(t p) c -> p t c", p=128),
                    o_sb[:].rearrange("p (t c) -> p t c", c=128))

            def dump_debug():
                # debug: dump raw attn (or zeros) so outputs are produced
                for tt in range(32):
                    o_sb = loup.tile([128, 128], F32, tag="od",
                                     name=f"od_{tt}")
                    if KSTAGE >= 18:
                        nc.vector.tensor_copy(
                            o_sb[:], attn_sb[:, tt * 128:(tt + 1) * 128])
                    else:
                        nc.vector.memset(o_sb[:], 0.0)
                    nc.sync.dma_start(out_d[tt * 128:(tt + 1) * 128, :],
                                      o_sb[:])

            if KSTAGE >= 17:
                pend = None
                for u in units:
                    pt_pair = stage1(u, pend)
                    if KSTAGE >= 26 and pend is not None \
                            and pend[0] == (0, NQB - 1):
                        stats_flush(0)
                    pend = (u, pt_pair)
                stage1(None, pend)
                if KSTAGE >= 26:
                    stats_flush(1)
            elif KSTAGE >= 15:
                # scores + exp only: no AV/stage2/stats
                pend = None
                for u in units:
                    stage1(u, None)
            if KSTAGE >= 30:
                # Both LN halves run in-stream: batch-0's overlaps
                # batch-1's AllGather (gather-0 done long ago), and
                # batch-1's is terminal work -- nothing left in the engine
                # queues to block, so no critical section is needed.
                if KLN0:
                    ln_half(0, lnsp, loup)
                    ln_half(1, lnsp, loup)
                else:
                    with tc.tile_critical(name="ln_tail",
                                          no_gpsimd_drain=True):
                        with tile.TileContext(nc) as tc2:
                            with tc2.tile_pool(name="lnp2", bufs=1) as ln2, \
                                    tc2.tile_pool(name="outp2",
                                                  bufs=1) as ou2:
                                ln_half(0, ln2, ou2)
                                ln_half(1, ln2, ou2)
            else:
                dump_debug()

    nc.compile()
    return nc


_CACHE: dict = {}
LAST_EXEC_NS = None


def _rope_tables():
    half = DH // 2
    inv_freq = 1.0 / (ROPE_BASE ** (np.arange(half, dtype=np.float32) / half))
    t = np.arange(S, dtype=np.float32)
    freqs = t[:, None] * inv_freq[None, :]
    emb = np.concatenate([freqs, freqs], axis=-1)          # [S, DH]
    return np.cos(emb).astype(np.float32), np.sin(emb).astype(np.float32)


def prep_flags(inputs):
    b_qk = np.asarray(inputs["b_qk"], dtype=np.float32)
    b_v = np.asarray(inputs["b_v"], dtype=np.float32)
    gamma = np.asarray(inputs["ln_gamma"], dtype=np.float32)
    beta = np.asarray(inputs["ln_beta"], dtype=np.float32)
    return (bool(np.any(b_qk)), bool(np.any(b_v)),
            bool(np.any(gamma != 1.0) or np.any(beta != 0.0)))


def _perm_mat():
    Pm = np.zeros((128, 128), np.float32)
    for i in range(64):
        Pm[2 * i + 1, 2 * i] = -1.0
        Pm[2 * i, 2 * i + 1] = 1.0
    return Pm


def _prep_in_maps(inputs, flags):
    x_qk = np.asarray(inputs["x_qk"], dtype=np.float32)
    x_v = np.asarray(inputs["x_v"], dtype=np.float32)
    W_qk = np.asarray(inputs["W_qk"], dtype=np.float32)
    b_qk = np.asarray(inputs["b_qk"], dtype=np.float32)
    W_v = np.asarray(inputs["W_v"], dtype=np.float32)
    b_v = np.asarray(inputs["b_v"], dtype=np.float32)
    gamma = np.asarray(inputs["ln_gamma"], dtype=np.float32)
    beta = np.asarray(inputs["ln_beta"], dtype=np.float32)

    Pm = _perm_mat()
    Pm64 = Pm[:DH, :DH]
    cos_all, sin_all = _rope_tables()          # [S, 64]
    cos_in = np.ascontiguousarray(
        np.tile(cos_all.T, (2, 1)).astype(NP_BF16))  # [128, 2048]
    sin_in = np.ascontiguousarray(
        np.tile(sin_all.T, (2, 1)).astype(NP_BF16))

    Wq = W_qk[:, :D]
    Wk = W_qk[:, D:]
    bq = b_qk[:D].reshape(H, DH)
    bk = b_qk[D:].reshape(H, DH)
    bq2 = bq @ Pm64
    bk2 = bk @ Pm64

    def seg_tile(x):
        # [seg*128 + p, dc*SEGW + c] = x[seg*SEGW + c, dc*128 + p]
        return np.ascontiguousarray(
            x.reshape(NSEG, SEGW, DC, 128).transpose(0, 3, 2, 1)
            .reshape(NSEG * 128, DC * SEGW).astype(NP_BF16))

    xqT_np = seg_tile(x_qk.reshape(R, D))
    xvT_np = seg_tile(x_v.reshape(R, D))
    perm_np = np.ascontiguousarray(Pm.astype(NP_BF16))
    ident_np = np.ascontiguousarray(np.eye(128, dtype=NP_BF16))

    in_maps = []
    for c in range(NC):
        cols = slice(c * 128, (c + 1) * 128)
        m = {
            "xqT": xqT_np, "xvT": xvT_np,
            "wq": np.ascontiguousarray(Wq[:, cols].astype(NP_BF16)),
            "wk": np.ascontiguousarray(Wk[:, cols].astype(NP_BF16)),
            "wv": np.ascontiguousarray(W_v[:, cols].astype(NP_BF16)),
            "perm": perm_np, "ident": ident_np,
            "cos": cos_in, "sin": sin_in,
        }
        if flags[0]:
            # additive post-RoPE bias tables for this head pair
            def fold(bh, bh2):
                rows = [bh[2 * c + hl][:, None] * cos_all.T
                        + bh2[2 * c + hl][:, None] * sin_all.T
                        for hl in range(2)]          # each [64, S]
                return np.ascontiguousarray(
                    np.tile(np.vstack(rows), (1, 2)).astype(np.float32))
            m["cq"] = fold(bq, bq2)
            m["ck"] = fold(bk, bk2)
        if flags[1]:
            m["bv"] = np.ascontiguousarray(
                b_v[c * 128:(c + 1) * 128].astype(np.float32)
                .reshape(128, 1))
        if flags[2]:
            m["gamma"] = np.ascontiguousarray(np.broadcast_to(
                gamma[c * 128:(c + 1) * 128], (128, 128)).astype(np.float32))
            m["beta"] = np.ascontiguousarray(np.broadcast_to(
                beta[c * 128:(c + 1) * 128], (128, 128)).astype(np.float32))
        in_maps.append(m)
    return in_maps


def kernel(**inputs):
    flags = prep_flags(inputs)
    if flags not in _CACHE:
        _CACHE[flags] = _build(flags)
    nc = _CACHE[flags]
    in_maps = _prep_in_maps(inputs, flags)
    res = bass_utils.run_bass_kernel_spmd(
        nc, in_maps, core_ids=list(range(NC)))
    global LAST_EXEC_NS
    LAST_EXEC_NS = res.exec_time_ns
    out = np.empty((R, D), np.float32)
    for c in range(NC):
        out[:, c * 128:(c + 1) * 128] = np.asarray(
            res.results[c]["out"], dtype=np.float32)
    return out.reshape(B, S, D)
